# revision 1
# baseline (speedup 1.0000x reference)
"""Trainium2 Bass kernel for nn_Linear_act_sp (2:4 activation-sparse linear).

Math (reference):
    max_act = max|x| over rows            [in]
    max_w   = max|W| over out rows        [in]
    s       = sqrt(max_act / clip(max_w)) [in]
    x_sp    = top2-of-4-magnitude prune of (x / s)
    out     = x_sp @ (W * s).T

Key identity: (x/s * mask) * s == x * mask, so out = (x * mask) @ W.T where
the mask keeps the top-2 of |x/s| within each contiguous group of 4 along
`in`.

Single-launch implementation (8 NeuronCores, data-parallel over rows):
  * Host permutes the contraction dim: in' = j*1024 + g (orig 4g+j) for both
    x and W.T.  A 2:4 group then occupies the SAME partition in four k-tile
    column blocks of the transposed activation -- the top-2 mask becomes
    pure elementwise work in the transposed domain with r=1/s a per-partition
    scalar, and the PE transposes run BEFORE the stats are known.
  * Phase 0: W.T stat slice (free-dim abs-max reduce), x row shard DMA'd,
    PE-transposed into full-f32 lhsT layout via PSUM -> scratch -> SBUF DMA
    (DMA preserves bits; mask later compares full-precision values), stats
    reduced from the transposed scratch.
  * 32 KB AllGather + 7 local max combines (cheaper than AllReduce) gives
    global stats; r = sqrt(clip(max_w)/max_act) via Newton-refined
    reciprocal + sqrt (~1 ulp, so mask ties track the f32 reference).
  * Mask per k0-chunk: ACT computes v=|x*r| (per-partition scale), DVE
    min/max tree -> threshold, GpSimd is_ge, DVE applies the mask writing
    f32r-rounded values in place (the single rounding the f32r matmul needs).
  * f32r matmul stream at N=512, contraction ordered so it consumes chunks
    as they are masked; LDWEIGHTS hidden behind the stream; PSUM drained by
    ACT/DVE split to overlap o-block transitions.
"""

import numpy as np

import concourse.bacc as bacc
import concourse.tile as tile
from concourse import mybir
from concourse.bass_utils import run_bass_kernel_spmd

AluOpType = mybir.AluOpType
ACTF = mybir.ActivationFunctionType

N_CORES = 8
N_ROWS = 8192          # 4*2048
D_IN = 4096
D_OUT = 4096
ROWS_PER_CORE = N_ROWS // N_CORES      # 1024
P = 128
EPS = np.float32(1e-8)

F32 = mybir.dt.float32
F32R = mybir.dt.float32r

NT = ROWS_PER_CORE // P    # 8 row tiles
KT = D_IN // P             # 32 contraction tiles
OT = D_OUT // 512          # 8 output column blocks
# contraction order: mask chunk k0 yields k-tiles {k0, k0+8, k0+16, k0+24}
KSEQ = [k0 + 8 * j for k0 in range(8) for j in range(4)]

_cache = {}

# test.py introspection: list of BassKernelResults from the last kernel() call
last_results = []


def _build():
    nc = bacc.Bacc("TRN2", target_bir_lowering=False, debug=False,
                   num_devices=N_CORES)
    xs = nc.dram_tensor("xs", [ROWS_PER_CORE, D_IN], F32, kind="ExternalInput")
    wt_d = nc.dram_tensor("wt", [D_IN, D_OUT], F32R, kind="ExternalInput")
    ws_d = nc.dram_tensor("ws", [D_IN, 512], F32, kind="ExternalInput")
    ident = nc.dram_tensor("ident", [P, P], F32, kind="ExternalInput")
    ys = nc.dram_tensor("ys", [ROWS_PER_CORE, D_OUT], F32, kind="ExternalOutput")

    with tile.TileContext(nc) as tc:
        with tc.tile_pool(name="const", bufs=1) as cpool, \
             tc.tile_pool(name="xmT", bufs=1) as xpool, \
             tc.tile_pool(name="xin", bufs=3) as xin, \
             tc.tile_pool(name="scr", bufs=3) as scr, \
             tc.tile_pool(name="wst", bufs=2) as wst, \
             tc.tile_pool(name="sml", bufs=2) as sml, \
             tc.tile_pool(name="vv", bufs=4) as vpool, \
             tc.tile_pool(name="tt", bufs=3) as tpool, \
             tc.tile_pool(name="thr", bufs=1) as thrp, \
             tc.tile_pool(name="wts", bufs=4) as wpool, \
             tc.tile_pool(name="outs", bufs=2) as opool, \
             tc.tile_pool(name="dram", bufs=2, space="DRAM") as dpool, \
             tc.tile_pool(name="psum", bufs=8, space="PSUM") as psum:
            id_t = cpool.tile([P, P], F32, tag="ident")
            nc.sync.dma_start(id_t[:], ident.ap()[:, :])
            # transposed activations (full f32 until masked), k-major:
            # block (k, n) at xmT[:, k*1024 + n*128 : +128]
            xmT = xpool.tile([P, KT * ROWS_PER_CORE], F32, tag="xmT")
            xmT3 = xmT[:].rearrange("p (k c) -> p k c", c=ROWS_PER_CORE)
            # local stats: cols 0..31 = max|x| per (p, k); 32..63 = max|W|
            stats = cpool.tile([P, 2 * KT], F32, tag="stats")

            # ---- x transpose pipeline with W-stat DMAs interleaved on the
            # sync ring (1 W-stat DMA per 2 x quarters; the W reduces pace
            # ahead of their gated DMAs so the ring never stalls long) ----
            ws3 = ws_d.ap()[:, :].rearrange("(k p) c -> p k c", p=P)

            def w_stat(g):
                wt_t = wst.tile([P, 1024], F32, tag="ws", name=f"ws{g}")
                nc.sync.dma_start(
                    wt_t[:].rearrange("p (k c) -> p k c", c=512),
                    ws3[:, 2 * g:2 * g + 2, :])
                nc.vector.tensor_reduce(
                    stats[:, KT + 2 * g:KT + 2 * g + 2],
                    wt_t[:].rearrange("p (k c) -> p k c", c=512),
                    axis=mybir.AxisListType.X,
                    op=AluOpType.max, apply_absolute_value=True)

            for n in range(NT):
                for hq in range(4):
                    xt = xin.tile([P, 1024], F32, tag="xt",
                                  name=f"xt{n}_{hq}")
                    c0 = hq * 1024
                    nc.sync.dma_start(
                        xt[:], xs.ap()[n * P:(n + 1) * P, c0:c0 + 1024])
                    if (n * 4 + hq) % 2 == 0:
                        w_stat((n * 4 + hq) // 2)
                    sc = scr.tile([P, 1024], F32, tag="sc", name=f"sc{n}_{hq}")
                    for g in range(2):
                        ps = psum.tile([P, 512], F32, tag="ps",
                                       name=f"tp{n}_{hq}_{g}")
                        for j in range(4):
                            c = g * 512 + j * P
                            nc.tensor.transpose(ps[:, j * P:(j + 1) * P],
                                                xt[:, c:c + P], id_t[:])
                        nc.scalar.activation(
                            sc[:, g * 512:(g + 1) * 512], ps[:], ACTF.Copy)
                    sc3 = sc[:].rearrange("p (kl c) -> p kl c", c=P)
                    if n == 0:
                        nc.vector.tensor_reduce(
                            stats[:, hq * 8:(hq + 1) * 8], sc3,
                            axis=mybir.AxisListType.X,
                            op=AluOpType.max, apply_absolute_value=True)
                    else:
                        tmp = sml.tile([P, 8], F32, tag="tmp",
                                       name=f"tm{n}_{hq}")
                        nc.vector.tensor_reduce(
                            tmp[:], sc3, axis=mybir.AxisListType.X,
                            op=AluOpType.max, apply_absolute_value=True)
                        nc.vector.tensor_tensor(
                            stats[:, hq * 8:(hq + 1) * 8],
                            stats[:, hq * 8:(hq + 1) * 8],
                            tmp[:], op=AluOpType.max)
                    # full-bit SBUF->SBUF DMA into the f32r-consumed buffer
                    # (f32 APs: an f32r-typed DMA would round to f32r's
                    # reduced mantissa and widen the mask tie-flip window;
                    # scalar ring: its wait-on-ACT must not stall the x
                    # input stream on the sync ring)
                    nc.scalar.dma_start(
                        xmT3[:, hq * 8:(hq + 1) * 8, n * P:(n + 1) * P],
                        sc3)

            # ---- AllReduce(max) of the [128, 64] stats ----
            bi = dpool.tile([P, 2 * KT], F32, tag="bi")
            bo = dpool.tile([P, 2 * KT], F32, tag="bo")
            nc.gpsimd.dma_start(bi[:], stats[:])
            nc.gpsimd.collective_compute(
                "AllReduce", AluOpType.max,
                replica_groups=[list(range(N_CORES))],
                ins=[bi[:].opt()], outs=[bo[:].opt()])
            g8 = cpool.tile([P, 2 * KT], F32, tag="g8")
            nc.gpsimd.dma_start(g8[:], bo[:])

            # ---- r = sqrt(clip(max_w, eps) / max_act), Newton-refined ----
            ma = g8[:, 0:KT]
            mw = g8[:, KT:2 * KT]
            inv = cpool.tile([P, KT], F32, tag="inv")
            nc.vector.reciprocal(inv[:], ma)
            t0 = cpool.tile([P, KT], F32, tag="t0")
            nc.vector.tensor_mul(t0[:], ma, inv[:])
            nc.vector.tensor_scalar(t0[:], t0[:], -1.0, 2.0,
                                    op0=AluOpType.mult, op1=AluOpType.add)
            nc.vector.tensor_mul(inv[:], inv[:], t0[:])
            wc = cpool.tile([P, KT], F32, tag="wc")
            nc.vector.tensor_scalar_max(wc[:], mw, float(EPS))
            q_t = cpool.tile([P, KT], F32, tag="q")
            nc.vector.tensor_mul(q_t[:], wc[:], inv[:])
            rr = cpool.tile([P, KT], F32, tag="rr")
            nc.scalar.activation(rr[:], q_t[:], ACTF.Sqrt)
            rec = cpool.tile([P, KT], F32, tag="rec")
            nc.vector.reciprocal(rec[:], rr[:])
            nc.vector.tensor_mul(rec[:], q_t[:], rec[:])
            nc.vector.tensor_tensor(rr[:], rr[:], rec[:], op=AluOpType.add)
            nc.vector.tensor_scalar_mul(rr[:], rr[:], 0.5)

            # ---- mask: top-2 of |x|*r within each quad, in place ----
            for k0 in range(8):
                v = []
                for j in range(4):
                    k = 8 * j + k0
                    vt = vpool.tile([P, 1024], F32, tag="v",
                                    name=f"v{k0}_{j}")
                    nc.scalar.activation(vt[:], xmT3[:, k, :], ACTF.Abs,
                                         scale=rr[:, k:k + 1])
                    v.append(vt)
                t1 = tpool.tile([P, 1024], F32, tag="t", name=f"t1_{k0}")
                t2 = tpool.tile([P, 1024], F32, tag="t", name=f"t2_{k0}")
                t3 = tpool.tile([P, 1024], F32, tag="t", name=f"t3_{k0}")
                nc.vector.tensor_max(t1[:], v[0][:], v[1][:])
                nc.vector.tensor_tensor(t2[:], v[0][:], v[1][:],
                                        op=AluOpType.min)
                nc.vector.tensor_max(t3[:], v[2][:], v[3][:])
                nc.vector.tensor_tensor(t1[:], t1[:], t3[:], op=AluOpType.min)
                nc.vector.tensor_tensor(t3[:], v[2][:], v[3][:],
                                        op=AluOpType.min)
                nc.vector.tensor_max(t2[:], t2[:], t3[:])
                thr = thrp.tile([P, 1024], F32, tag="thr", name=f"thr{k0}")
                nc.vector.tensor_max(thr[:], t1[:], t2[:])
                for j in range(4):
                    k = 8 * j + k0
                    nc.vector.tensor_tensor(v[j][:], v[j][:], thr[:],
                                            op=AluOpType.is_ge)
                    nc.vector.tensor_tensor(
                        xmT3[:, k, :].bitcast(F32R), xmT3[:, k, :],
                        v[j][:], op=AluOpType.mult)

            # ---- matmul stream: out = xmT.T @ W.T ----
            for o in range(OT):
                psn = {n: psum.tile([P, 512], F32, tag="ps",
                                    name=f"mm{o}_{n}")
                       for n in range(NT)}
                for ki, k in enumerate(KSEQ):
                    w_t = wpool.tile([P, 512], F32R, tag="wt",
                                     name=f"w{o}_{k}")
                    nc.sync.dma_start(
                        w_t[:],
                        wt_d.ap()[k * P:(k + 1) * P, o * 512:(o + 1) * 512])
                    for n in range(NT):
                        nc.tensor.matmul(
                            psn[n][:],
                            xmT3[:, k, n * P:(n + 1) * P].bitcast(F32R),
                            w_t[:],
                            start=(ki == 0), stop=(ki == KT - 1))
                for n in range(NT):
                    ot = opool.tile([P, 512], F32, tag="ot",
                                    name=f"ot{o}_{n}")
                    if o >= 2 and n >= 4:
                        nc.vector.tensor_copy(ot[:], psn[n][:])
                    else:
                        nc.scalar.activation(ot[:], psn[n][:], ACTF.Copy)
                    nc.sync.dma_start(
                        ys.ap()[n * P:(n + 1) * P, o * 512:(o + 1) * 512],
                        ot[:])
    nc.compile()
    return nc


def _get():
    if "main" not in _cache:
        _cache["main"] = _build()
    return _cache["main"]


# contraction-dim permutation: new col j*1024+g holds orig col 4g+j
_PERM = np.arange(D_IN).reshape(D_IN // 4, 4).T.reshape(-1)


def kernel(x: np.ndarray, W: np.ndarray) -> np.ndarray:
    global last_results
    last_results = []
    bs, seq, d_in = x.shape
    xf = x.reshape(-1, d_in).astype(np.float32, copy=False)
    W = np.asarray(W, dtype=np.float32)

    xp = np.ascontiguousarray(xf[:, _PERM])
    wtp = np.ascontiguousarray(W.T[_PERM, :])
    ident = np.eye(P, dtype=np.float32)

    nc = _get()
    in_maps = []
    for c in range(N_CORES):
        in_maps.append({
            "xs": np.ascontiguousarray(
                xp[c * ROWS_PER_CORE:(c + 1) * ROWS_PER_CORE]),
            "wt": wtp,
            "ws": np.ascontiguousarray(wtp[:, c * 512:(c + 1) * 512]),
            "ident": ident,
        })
    res = run_bass_kernel_spmd(nc, in_maps, list(range(N_CORES)))
    last_results.append(res)

    out = np.concatenate([res.results[c]["ys"] for c in range(N_CORES)],
                         axis=0)
    return out.reshape(bs, seq, D_OUT)



# revision 4
# speedup vs baseline: 1.0747x; 1.0747x over previous
"""Trainium2 Bass kernel for nn_Linear_act_sp (2:4 activation-sparse linear).

Math (reference):
    max_act = max|x| over rows            [in]
    max_w   = max|W| over out rows        [in]
    s       = sqrt(max_act / clip(max_w)) [in]
    x_sp    = top2-of-4-magnitude prune of (x / s)
    out     = x_sp @ (W * s).T

Key identity: (x/s * mask) * s == x * mask, so out = (x * mask) @ W.T where
the mask keeps the top-2 of |x/s| within each contiguous group of 4 along
`in`.

Single-launch implementation (8 NeuronCores, data-parallel over rows):
  * Host permutes the contraction dim: in' = j*1024 + g (orig 4g+j) for both
    x and W.T.  A 2:4 group then occupies the SAME partition in four k-tile
    column blocks of the transposed activation -- the top-2 mask becomes
    pure elementwise work in the transposed domain with r=1/s a per-partition
    scalar, and the PE transposes run BEFORE the stats are known.
  * Phase 0: W.T stat slice (free-dim abs-max reduce), x row shard DMA'd,
    PE-transposed into full-f32 lhsT layout; ACT drains PSUM straight into
    the strided xmT destination (no scratch, no SBUF->SBUF DMA); stats
    reduced from xmT.
  * 32 KB AllReduce(max) gives global stats; r = sqrt(clip(max_w)/max_act)
    via Newton-refined reciprocal + sqrt (~1 ulp, so mask ties track the
    f32 reference).
  * Mask per k0-chunk: ACT computes v=|x*r| (per-partition scale); the
    min/max tree, is_ge and apply all run as scalar_tensor_tensor (2-port
    2x DVE mode).  The apply writes fp16 masked values IN PLACE into the
    low half of each f32 k-tile (trailing write, race-free); fp16 keeps
    rel err ~3e-4 (mask decisions are made in full f32).
  * fp16 matmul stream at N=512 (fp16 W streamed from DRAM, half the f32r
    traffic), contraction ordered so it consumes chunks as they are masked;
    per-bank PSUM drain issued right after each row-tile's last matmul,
    split ACT/Pool so DVE stays on mask production; outputs DMA'd from the
    drain engine's own ring.
"""

import numpy as np

import concourse.bacc as bacc
import concourse.tile as tile
from concourse import mybir
from concourse.bass_utils import run_bass_kernel_spmd

AluOpType = mybir.AluOpType
ACTF = mybir.ActivationFunctionType

N_CORES = 8
N_ROWS = 8192          # 4*2048
D_IN = 4096
D_OUT = 4096
ROWS_PER_CORE = N_ROWS // N_CORES      # 1024
P = 128
EPS = np.float32(1e-8)

F32 = mybir.dt.float32
F16 = mybir.dt.float16

NT = ROWS_PER_CORE // P    # 8 row tiles
KT = D_IN // P             # 32 contraction tiles
OT = D_OUT // 512          # 8 output column blocks
# contraction order: mask chunk k0 yields k-tiles {k0, k0+8, k0+16, k0+24}
KSEQ = [k0 + 8 * j for k0 in range(8) for j in range(4)]

_cache = {}

# test.py introspection: list of BassKernelResults from the last kernel() call
last_results = []


def _build():
    nc = bacc.Bacc("TRN2", target_bir_lowering=False, debug=False,
                   num_devices=N_CORES)
    xs = nc.dram_tensor("xs", [ROWS_PER_CORE, D_IN], F32, kind="ExternalInput")
    wt_d = nc.dram_tensor("wt", [D_IN, D_OUT], F16, kind="ExternalInput")
    ws_d = nc.dram_tensor("ws", [D_IN, 512], F32, kind="ExternalInput")
    ident = nc.dram_tensor("ident", [P, P], F32, kind="ExternalInput")
    ys = nc.dram_tensor("ys", [ROWS_PER_CORE, D_OUT], F32, kind="ExternalOutput")

    def stt(eng, out, a, b, op1):
        # (a * 1.0) op1 b  -- InstTensorScalarPtr, 2x 2-port DVE mode
        eng.scalar_tensor_tensor(out, a, 1.0, b, op0=AluOpType.mult, op1=op1)

    with tile.TileContext(nc) as tc:
        with tc.tile_pool(name="const", bufs=1) as cpool, \
             tc.tile_pool(name="xmT", bufs=1) as xpool, \
             tc.tile_pool(name="xin", bufs=2) as xin, \
             tc.tile_pool(name="wst", bufs=2) as wst, \
             tc.tile_pool(name="sml", bufs=2) as sml, \
             tc.tile_pool(name="vv", bufs=8) as vpool, \
             tc.tile_pool(name="tt", bufs=3) as tpool, \
             tc.tile_pool(name="wts", bufs=4) as wpool, \
             tc.tile_pool(name="outs", bufs=4) as opool, \
             tc.tile_pool(name="dram", bufs=2, space="DRAM") as dpool, \
             tc.tile_pool(name="psum", bufs=8, space="PSUM") as psum:
            id_t = cpool.tile([P, P], F32, tag="ident")
            nc.sync.dma_start(id_t[:], ident.ap()[:, :])
            # transposed activations (full f32 until masked), k-major:
            # block (k, n) at xmT[:, k*1024 + n*128 : +128]
            xmT = xpool.tile([P, KT * ROWS_PER_CORE], F32, tag="xmT")
            xmT3 = xmT[:].rearrange("p (k c) -> p k c", c=ROWS_PER_CORE)
            # fp16 view of the same storage: masked k-tile k lives at
            # xh3[:, k, 0:1024] (low half of the f32 k-tile's bytes)
            xh3 = xmT[:].bitcast(F16).rearrange("p (k c) -> p k c",
                                                c=2 * ROWS_PER_CORE)
            # local stats: cols 0..31 = max|x| per (p, k); 32..63 = max|W|
            stats = cpool.tile([P, 2 * KT], F32, tag="stats")

            # ---- x transpose pipeline with W-stat DMAs interleaved on the
            # sync ring ----
            ws3 = ws_d.ap()[:, :].rearrange("(k p) c -> p k c", p=P)

            def w_stat(g):
                wt_t = wst.tile([P, 1024], F32, tag="ws", name=f"ws{g}")
                nc.sync.dma_start(
                    wt_t[:].rearrange("p (k c) -> p k c", c=512),
                    ws3[:, 2 * g:2 * g + 2, :])
                nc.vector.tensor_reduce(
                    stats[:, KT + 2 * g:KT + 2 * g + 2],
                    wt_t[:].rearrange("p (k c) -> p k c", c=512),
                    axis=mybir.AxisListType.X,
                    op=AluOpType.max, apply_absolute_value=True)

            for n in range(NT):
                for hq in range(4):
                    xt = xin.tile([P, 1024], F32, tag="xt",
                                  name=f"xt{n}_{hq}")
                    c0 = hq * 1024
                    nc.sync.dma_start(
                        xt[:], xs.ap()[n * P:(n + 1) * P, c0:c0 + 1024])
                    if (n * 4 + hq) % 2 == 0:
                        w_stat((n * 4 + hq) // 2)
                    for g in range(2):
                        ps = psum.tile([P, 512], F32, tag="ps",
                                       name=f"tp{n}_{hq}_{g}")
                        for j in range(4):
                            c = g * 512 + j * P
                            nc.tensor.transpose(ps[:, j * P:(j + 1) * P],
                                                xt[:, c:c + P], id_t[:])
                        # drain PSUM straight into the final strided layout
                        k0 = hq * 8 + g * 4
                        nc.scalar.activation(
                            xmT3[:, k0:k0 + 4, n * P:(n + 1) * P],
                            ps[:].rearrange("p (a c) -> p a c", c=P),
                            ACTF.Copy)
                    src = xmT3[:, hq * 8:(hq + 1) * 8, n * P:(n + 1) * P]
                    if n == 0:
                        nc.vector.tensor_reduce(
                            stats[:, hq * 8:(hq + 1) * 8], src,
                            axis=mybir.AxisListType.X,
                            op=AluOpType.max, apply_absolute_value=True)
                    else:
                        tmp = sml.tile([P, 8], F32, tag="tmp",
                                       name=f"tm{n}_{hq}")
                        nc.vector.tensor_reduce(
                            tmp[:], src, axis=mybir.AxisListType.X,
                            op=AluOpType.max, apply_absolute_value=True)
                        nc.vector.tensor_tensor(
                            stats[:, hq * 8:(hq + 1) * 8],
                            stats[:, hq * 8:(hq + 1) * 8],
                            tmp[:], op=AluOpType.max)

            # ---- AllReduce(max) of the [128, 64] stats ----
            bi = dpool.tile([P, 2 * KT], F32, tag="bi")
            bo = dpool.tile([P, 2 * KT], F32, tag="bo")
            nc.gpsimd.dma_start(bi[:], stats[:])
            nc.gpsimd.collective_compute(
                "AllReduce", AluOpType.max,
                replica_groups=[list(range(N_CORES))],
                ins=[bi[:].opt()], outs=[bo[:].opt()])
            g8 = cpool.tile([P, 2 * KT], F32, tag="g8")
            nc.gpsimd.dma_start(g8[:], bo[:])

            # ---- r = sqrt(clip(max_w, eps) / max_act), Newton-refined ----
            ma = g8[:, 0:KT]
            mw = g8[:, KT:2 * KT]
            inv = cpool.tile([P, KT], F32, tag="inv")
            nc.vector.reciprocal(inv[:], ma)
            t0 = cpool.tile([P, KT], F32, tag="t0")
            nc.vector.tensor_mul(t0[:], ma, inv[:])
            nc.vector.tensor_scalar(t0[:], t0[:], -1.0, 2.0,
                                    op0=AluOpType.mult, op1=AluOpType.add)
            nc.vector.tensor_mul(inv[:], inv[:], t0[:])
            wc = cpool.tile([P, KT], F32, tag="wc")
            nc.vector.tensor_scalar_max(wc[:], mw, float(EPS))
            q_t = cpool.tile([P, KT], F32, tag="q")
            nc.vector.tensor_mul(q_t[:], wc[:], inv[:])
            rr = cpool.tile([P, KT], F32, tag="rr")
            nc.scalar.activation(rr[:], q_t[:], ACTF.Sqrt)
            rec = cpool.tile([P, KT], F32, tag="rec")
            nc.vector.reciprocal(rec[:], rr[:])
            nc.vector.tensor_mul(rec[:], q_t[:], rec[:])
            nc.vector.tensor_tensor(rr[:], rr[:], rec[:], op=AluOpType.add)
            nc.vector.tensor_scalar_mul(rr[:], rr[:], 0.5)

            # ---- mask: top-2 of |x|*r within each quad; fp16 in place ----
            for k0 in range(8):
                v = []
                for j in range(4):
                    k = 8 * j + k0
                    vt = vpool.tile([P, 1024], F32, tag="v",
                                    name=f"v{k0}_{j}")
                    nc.scalar.activation(vt[:], xmT3[:, k, :], ACTF.Abs,
                                         scale=rr[:, k:k + 1])
                    v.append(vt)
                t1 = tpool.tile([P, 1024], F32, tag="t", name=f"t1_{k0}")
                t2 = tpool.tile([P, 1024], F32, tag="t", name=f"t2_{k0}")
                t3 = tpool.tile([P, 1024], F32, tag="t", name=f"t3_{k0}")
                # 2nd-largest of {v0..v3}:
                #   max(min(max01, max23), max(min01, min23))
                stt(nc.vector, t1[:], v[0][:], v[1][:], AluOpType.max)
                stt(nc.vector, t2[:], v[0][:], v[1][:], AluOpType.min)
                stt(nc.vector, t3[:], v[2][:], v[3][:], AluOpType.max)
                stt(nc.vector, t1[:], t1[:], t3[:], AluOpType.min)
                stt(nc.vector, t3[:], v[2][:], v[3][:], AluOpType.min)
                stt(nc.vector, t2[:], t2[:], t3[:], AluOpType.max)
                stt(nc.vector, t1[:], t1[:], t2[:], AluOpType.max)
                for j in range(4):
                    k = 8 * j + k0
                    stt(nc.vector, v[j][:], v[j][:], t1[:], AluOpType.is_ge)
                    stt(nc.vector, xh3[:, k, 0:ROWS_PER_CORE], v[j][:],
                        xmT3[:, k, :], AluOpType.mult)

            # ---- fp16 matmul stream: out = x_spT.T @ W.T ----
            for o in range(OT):
                psn = {n: psum.tile([P, 512], F32, tag="ps",
                                    name=f"mm{o}_{n}")
                       for n in range(NT)}
                for ki, k in enumerate(KSEQ):
                    w_t = wpool.tile([P, 512], F16, tag="wt",
                                     name=f"w{o}_{k}")
                    nc.sync.dma_start(
                        w_t[:],
                        wt_d.ap()[k * P:(k + 1) * P, o * 512:(o + 1) * 512])
                    last = ki == KT - 1
                    for n in range(NT):
                        nc.tensor.matmul(
                            psn[n][:],
                            xh3[:, k, n * P:(n + 1) * P],
                            w_t[:],
                            start=(ki == 0), stop=last)
                        if last:
                            ot = opool.tile([P, 512], F32, tag="ot",
                                            name=f"ot{o}_{n}")
                            if n % 2 == 0:
                                nc.scalar.activation(ot[:], psn[n][:],
                                                     ACTF.Copy)
                                nc.scalar.dma_start(
                                    ys.ap()[n * P:(n + 1) * P,
                                            o * 512:(o + 1) * 512],
                                    ot[:])
                            else:
                                nc.vector.tensor_copy(ot[:], psn[n][:])
                                nc.gpsimd.dma_start(
                                    ys.ap()[n * P:(n + 1) * P,
                                            o * 512:(o + 1) * 512],
                                    ot[:])
    nc.compile()
    return nc


def _get():
    if "main" not in _cache:
        _cache["main"] = _build()
    return _cache["main"]


# contraction-dim permutation: new col j*1024+g holds orig col 4g+j
_PERM = np.arange(D_IN).reshape(D_IN // 4, 4).T.reshape(-1)


def kernel(x: np.ndarray, W: np.ndarray) -> np.ndarray:
    global last_results
    last_results = []
    bs, seq, d_in = x.shape
    xf = x.reshape(-1, d_in).astype(np.float32, copy=False)
    W = np.asarray(W, dtype=np.float32)

    xp = np.ascontiguousarray(xf[:, _PERM])
    wtp = np.ascontiguousarray(W.T[_PERM, :])
    wtp16 = wtp.astype(np.float16)
    ident = np.eye(P, dtype=np.float32)

    nc = _get()
    in_maps = []
    for c in range(N_CORES):
        in_maps.append({
            "xs": np.ascontiguousarray(
                xp[c * ROWS_PER_CORE:(c + 1) * ROWS_PER_CORE]),
            "wt": wtp16,
            "ws": np.ascontiguousarray(wtp[:, c * 512:(c + 1) * 512]),
            "ident": ident,
        })
    res = run_bass_kernel_spmd(nc, in_maps, list(range(N_CORES)))
    last_results.append(res)

    out = np.concatenate([res.results[c]["ys"] for c in range(N_CORES)],
                         axis=0)
    return out.reshape(bs, seq, D_OUT)
